# revision 26
# baseline (speedup 1.0000x reference)
"""Trainium2 Bass kernel for nn_AttenBlock (sparse_attention).

Strategy: data-parallel over batch across 8 NeuronCores (4 batches/core).
Per core, a feature-major bf16 pipeline:
  LN(x) -> conv1d(last_x) -> softmax attention over 3-tap windows ->
  coarse/fine window mix -> SwiGLU -> down_proj -> token_proj.
All matmuls run on the PE array in bf16 (fp32 PSUM accumulation) with
moving free dim 512. Host pre-transposes activations/weights into the
SBUF layouts so no on-device transposes are needed.

Schedule: software-pipelined across batches (front of b+1 interleaves
with the SwiGLU/token phase of b). Within the front, conv runs first
(depends only on last_x) so the PE fills early at kernel start; for
batch 0 the heavy phase of chunk 0 starts as soon as chunk 0's features
are ready, while the rest of the front interleaves. The SwiGLU streams
gate/up weights once per i-tile shared by both 512-token chunks
(chunk 1's h tiles are kept in SBUF and its down_proj runs as a tail).
LayerNorm output is written in place over the x tile.

Token stream layout per batch: 1026 columns = [zero | 1021 tokens | 3
pad + zero edge]. Pad columns produce garbage tokens whose token_proj
weights are zero, so they never contribute to the output; the one halo
column each side of the real tokens is zeroed so windows match the
reference's zero padding.
"""

import sys

sys.path.insert(0, "/opt/trn_rl_repo")

import numpy as np
import ml_dtypes

import concourse.bass as bass
import concourse.mybir as mybir
import concourse.tile as tile
from concourse import bacc
from concourse.bass_utils import run_bass_kernel_spmd

# problem shapes (hardcoded; harness provides matching inputs)
B, L, D = 32, 1021, 512
INTER, MOUT = 2048, 4096
NCORES = 8
BPC = B // NCORES        # batches per core
KT_D = D // 128          # 4  feature k-tiles
KT_2D = 2 * KT_D         # 8  feat k-tiles
NIT = INTER // 128       # 16 inter tiles
LP = 1026                # padded token columns per batch
CH = 512                 # token chunk
NCH = 2                  # chunks per batch
LTB = 8                  # 128-token l-tiles per batch
MT = MOUT // 128         # 32 output m-tiles

fp32 = mybir.dt.float32
CDT = mybir.dt.bfloat16
NPDT = ml_dtypes.bfloat16
AF = mybir.ActivationFunctionType

LAST_EXEC_NS = None
LAST_RESULTS = None
_NC_CACHE = None


def _emit(nc):
    x_t = nc.dram_tensor("x_t", [BPC, 128, KT_D, LP], CDT, kind="ExternalInput")
    lxin = nc.dram_tensor("lx_t", [BPC, 128, KT_D, LP], CDT, kind="ExternalInput")
    conv_wt = nc.dram_tensor("conv_wt", [128, KT_D, 3, D], CDT, kind="ExternalInput")
    atten_wt = nc.dram_tensor("atten_wt", [128, KT_D, 3], CDT, kind="ExternalInput")
    gate_wt = nc.dram_tensor("gate_wt", [128, NIT, KT_2D, 128], CDT, kind="ExternalInput")
    up_wt = nc.dram_tensor("up_wt", [128, NIT, KT_2D, 128], CDT, kind="ExternalInput")
    down_wt = nc.dram_tensor("down_wt", [128, NIT, D], CDT, kind="ExternalInput")
    tok_wt = nc.dram_tensor("tok_wt", [128, MT, LTB, 128], CDT, kind="ExternalInput")
    g_in = nc.dram_tensor("g_in", [128, KT_D], fp32, kind="ExternalInput")
    b_in = nc.dram_tensor("b_in", [128, KT_D], fp32, kind="ExternalInput")
    out_h = nc.dram_tensor("out", [BPC, MOUT, D], fp32, kind="ExternalOutput")

    from contextlib import ExitStack
    from itertools import chain

    with tile.TileContext(nc) as tc, ExitStack() as ctx:
        singles = ctx.enter_context(tc.tile_pool(name="singles", bufs=1))
        px = ctx.enter_context(tc.tile_pool(name="px", bufs=2))
        plx = ctx.enter_context(tc.tile_pool(name="plx", bufs=1))
        pfront = ctx.enter_context(tc.tile_pool(name="pfront", bufs=2))
        pbatt = ctx.enter_context(tc.tile_pool(name="pbatt", bufs=2))
        pfeat = ctx.enter_context(tc.tile_pool(name="pfeat", bufs=4))
        pstatf = ctx.enter_context(tc.tile_pool(name="pstatf", bufs=4))
        pstatc = ctx.enter_context(tc.tile_pool(name="pstatc", bufs=4))
        pstatb = ctx.enter_context(tc.tile_pool(name="pstatb", bufs=2))
        psq = ctx.enter_context(tc.tile_pool(name="psq", bufs=2))
        ptmpc = ctx.enter_context(tc.tile_pool(name="ptmpc", bufs=2))
        pgu = ctx.enter_context(tc.tile_pool(name="pgu", bufs=4))
        psg = ctx.enter_context(tc.tile_pool(name="psg", bufs=2))
        ph0 = ctx.enter_context(tc.tile_pool(name="ph0", bufs=2))
        ph1 = ctx.enter_context(tc.tile_pool(name="ph1", bufs=NIT))
        ptw = ctx.enter_context(tc.tile_pool(name="ptw", bufs=4))
        py_sb = ctx.enter_context(tc.tile_pool(name="py_sb", bufs=1))
        pout = ctx.enter_context(tc.tile_pool(name="pout", bufs=2))
        pgen = ctx.enter_context(tc.tile_pool(name="pgen", bufs=2, space="PSUM"))
        pfps = ctx.enter_context(tc.tile_pool(name="pfps", bufs=2, space="PSUM"))
        pyps = ctx.enter_context(tc.tile_pool(name="pyps", bufs=4, space="PSUM"))

        # ---- batch-0 inputs FIRST: the DMA engines drain in emission order,
        # so the conv of batch 0 (first PE work) must not queue behind the
        # resident-weight transfers.
        state = {}
        st0 = state[0] = {}
        lxt0 = plx.tile([128, KT_D, LP], CDT, tag="lxt", name="lxt0")
        nc.sync.dma_start(lxt0[:], lxin[0])
        conv_sb = singles.tile([128, KT_D, 3, D], CDT)
        nc.sync.dma_start(conv_sb[:], conv_wt[:])
        xt0 = px.tile([128, KT_D, LP], CDT, tag="xt", name="xt0")
        nc.sync.dma_start(xt0[:], x_t[0])
        st0["xt"], st0["lxt"] = xt0, lxt0

        atten_sb = singles.tile([128, KT_D, 3], CDT)
        nc.sync.dma_start(atten_sb[:], atten_wt[:])
        g_sb = singles.tile([128, KT_D], fp32)
        nc.sync.dma_start(g_sb[:], g_in[:])
        b_sb = singles.tile([128, KT_D], fp32)
        nc.sync.dma_start(b_sb[:], b_in[:])
        down_sb = singles.tile([128, NIT, D], CDT)
        nc.sync.dma_start(down_sb[:], down_wt[:])
        ones_col = singles.tile([128, 1], CDT)
        nc.vector.memset(ones_col[:], 1.0)
        ones_row = singles.tile([1, 128], CDT)
        nc.vector.memset(ones_row[:], 1.0)
        eps_t = singles.tile([1, 1], fp32)
        nc.vector.memset(eps_t[:], 1e-5)

        def conv_chunk(st, c):
            """3-tap conv for one 512-col chunk, one do-tile per yield."""
            lxt, lxT = st["lxt"], st["lxT"]
            for do in range(KT_D):
                cps = pfps.tile([128, CH], fp32, tag="fps", name="cps")
                idx = 0
                for tap in range(3):
                    for kin in range(KT_D):
                        nc.tensor.matmul(
                            cps[:],
                            conv_sb[:, kin, tap, do * 128:(do + 1) * 128],
                            lxt[:, kin, c * CH + tap:c * CH + tap + CH],
                            start=(idx == 0), stop=(idx == 11))
                        idx += 1
                nc.vector.tensor_copy(lxT[:, do, 1 + c * CH:1 + c * CH + CH], cps[:])
                yield

        def ln_stats(st, c):
            """LayerNorm sum / sum-of-squares matmuls for one chunk."""
            xt = st["xt"]
            sl = slice(1 + c * CH, 1 + c * CH + CH)
            s1 = pfps.tile([128, CH], fp32, tag="fps", name="s1")
            for kt in range(KT_D):
                nc.tensor.matmul(s1[:1], ones_col[:], xt[:, kt, sl],
                                 start=(kt == 0), stop=(kt == KT_D - 1))
            s2 = pfps.tile([128, CH], fp32, tag="fps", name="s2")
            for kt in range(KT_D):
                sq = psq.tile([128, CH], CDT, tag="sq", name="sq")
                nc.vector.tensor_mul(sq[:], xt[:, kt, sl], xt[:, kt, sl])
                nc.tensor.matmul(s2[:1], ones_col[:], sq[:],
                                 start=(kt == 0), stop=(kt == KT_D - 1))
            st[f"s12_{c}"] = (s1, s2)
            yield

        def ln_chain(st, c):
            """mean/rstd computation (vector/scalar only, no PE)."""
            s1, s2 = st.pop(f"s12_{c}")
            mean_c = pstatc.tile([1, CH], CDT, tag="stc", name="mean_c")
            nc.vector.tensor_scalar_mul(mean_c[:], s1[:1], 1.0 / D)
            msq = pstatf.tile([1, CH], fp32, tag="stf", name="msq")
            nc.vector.tensor_mul(msq[:], mean_c[:], mean_c[:])
            var = pstatf.tile([1, CH], fp32, tag="stf", name="var")
            nc.vector.tensor_scalar_mul(var[:], s2[:1], 1.0 / D)
            nc.vector.tensor_sub(var[:], var[:], msq[:])
            # rstd via Ln/Exp tables (Rsqrt/Reciprocal ACT tables are
            # blocked for accuracy; single-partition DVE recip is ~4us)
            lnv = pstatf.tile([1, CH], fp32, tag="stf", name="lnv")
            nc.scalar.activation(lnv[:], var[:], AF.Ln, bias=eps_t[:])
            rstd_c = pstatc.tile([1, CH], CDT, tag="stc", name="rstd_c")
            nc.scalar.activation(rstd_c[:], lnv[:], AF.Exp, scale=-0.5)
            st[f"mr_{c}"] = (mean_c, rstd_c)
            yield

        def ln_bcast(st, c):
            """broadcast mean/rstd across partitions; copy out of PSUM
            eagerly so the PSUM ring is decoupled from the apply reads."""
            mean_c, rstd_c = st.pop(f"mr_{c}")
            mb_ps = pfps.tile([128, CH], fp32, tag="fps", name="mb_ps")
            nc.tensor.matmul(mb_ps[:], ones_row[:], mean_c[:], start=True, stop=True)
            rb_ps = pfps.tile([128, CH], fp32, tag="fps", name="rb_ps")
            nc.tensor.matmul(rb_ps[:], ones_row[:], rstd_c[:], start=True, stop=True)
            mb_sb = pstatb.tile([128, CH], CDT, tag="mb", name="mb_sb")
            nc.scalar.copy(mb_sb[:], mb_ps[:])
            rb_sb = pstatb.tile([128, CH], CDT, tag="rb", name="rb_sb")
            nc.scalar.copy(rb_sb[:], rb_ps[:])
            st[f"mrb_{c}"] = (mb_sb, rb_sb)
            yield

        def ln_apply(st, c):
            """xn = (x - m) * r * g + b, written over xt (vector/scalar)."""
            xt = st["xt"]
            mb_sb, rb_sb = st.pop(f"mrb_{c}")
            sl = slice(1 + c * CH, 1 + c * CH + CH)
            for kt in range(KT_D):
                t = ptmpc.tile([128, CH], CDT, tag="lnt", name="lnt")
                nc.vector.tensor_sub(t[:], xt[:, kt, sl], mb_sb[:])
                t2 = ptmpc.tile([128, CH], CDT, tag="lnt", name="lnt2")
                nc.vector.tensor_mul(t2[:], t[:], rb_sb[:])
                nc.scalar.activation(xt[:, kt, sl], t2[:], AF.Identity,
                                     bias=b_sb[:, kt:kt + 1],
                                     scale=g_sb[:, kt:kt + 1])
            yield

        def smax_logits(st, c):
            """attention logits + exp for one chunk (PE + scalar)."""
            lxT = st["lxT"]
            sl = slice(1 + c * CH, 1 + c * CH + CH)
            lg_ps = pfps.tile([128, CH], fp32, tag="fps", name="lg_ps")
            for kin in range(KT_D):
                nc.tensor.matmul(lg_ps[:3], atten_sb[:, kin, :],
                                 lxT[:, kin, sl],
                                 start=(kin == 0), stop=(kin == KT_D - 1))
            exp_sb = pstatc.tile([3, CH], CDT, tag="exp3", name="exp_sb")
            nc.scalar.activation(exp_sb[:], lg_ps[:3], AF.Exp)
            expk = [pstatc.tile([1, CH], CDT, tag="expk", name=f"expk{k}")
                    for k in range(3)]
            for k in range(3):
                nc.sync.dma_start(expk[k][:], exp_sb[k:k + 1, :])
            st[f"expk_{c}"] = (exp_sb, expk)
            yield

        def smax_den(st, c):
            """1/sum(exp): 3-partition matmul reduce + Ln/Exp tables."""
            exp_sb, _ = st[f"expk_{c}"]
            den_ps = pfps.tile([128, CH], fp32, tag="fps", name="den_ps")
            nc.tensor.matmul(den_ps[:1], ones_col[:3], exp_sb[:],
                             start=True, stop=True)
            lnd = pstatf.tile([1, CH], fp32, tag="stf", name="lnd")
            nc.scalar.activation(lnd[:], den_ps[:1], AF.Ln)
            rec_c = pstatc.tile([1, CH], CDT, tag="stc", name="rec_c")
            nc.scalar.activation(rec_c[:], lnd[:], AF.Exp, scale=-1.0)
            st[f"rec_{c}"] = rec_c
            yield

        def smax_bcast(st, c):
            """broadcast denom + exp taps across partitions -> batt."""
            _, expk = st.pop(f"expk_{c}")
            rec_c = st.pop(f"rec_{c}")
            brec_ps = pfps.tile([128, CH], fp32, tag="fps", name="brec_ps")
            nc.tensor.matmul(brec_ps[:], ones_row[:], rec_c[:], start=True, stop=True)
            brec_sb = ptmpc.tile([128, CH], CDT, tag="brec", name="brec_sb")
            nc.scalar.copy(brec_sb[:], brec_ps[:])
            bt = st["batt_cs"][c]
            for k in range(3):
                bex_ps = pfps.tile([128, CH], fp32, tag="fps", name="bex_ps")
                nc.tensor.matmul(bex_ps[:], ones_row[:], expk[k][:], start=True, stop=True)
                nc.vector.tensor_mul(bt[:, k, :], bex_ps[:], brec_sb[:])
            yield

        def win_chunk(st, c):
            """windowed coarse/fine mix for one chunk -> feat."""
            lxT, xt = st["lxT"], st["xt"]
            bt = st["batt_cs"][c]
            ft = st["feat_cs"][c]
            for do in range(KT_D):
                t1 = ptmpc.tile([128, CH], CDT, tag="w1", name="w1")
                t2 = ptmpc.tile([128, CH], CDT, tag="w2", name="w2")
                nc.vector.tensor_mul(t1[:], bt[:, 0, :], lxT[:, do, c * CH:c * CH + CH])
                nc.vector.tensor_mul(t2[:], bt[:, 1, :], lxT[:, do, c * CH + 1:c * CH + 1 + CH])
                nc.vector.tensor_add(t1[:], t1[:], t2[:])
                nc.vector.tensor_mul(t2[:], bt[:, 2, :], lxT[:, do, c * CH + 2:c * CH + 2 + CH])
                nc.vector.tensor_add(ft[:, do, :], t1[:], t2[:])
                f1 = ptmpc.tile([128, CH], CDT, tag="f1", name="f1")
                f2 = ptmpc.tile([128, CH], CDT, tag="f2", name="f2")
                nc.vector.tensor_mul(f1[:], bt[:, 0, :], xt[:, do, c * CH:c * CH + CH])
                nc.vector.tensor_mul(f2[:], bt[:, 1, :], xt[:, do, c * CH + 1:c * CH + 1 + CH])
                nc.vector.tensor_add(f1[:], f1[:], f2[:])
                nc.vector.tensor_mul(f2[:], bt[:, 2, :], xt[:, do, c * CH + 2:c * CH + 2 + CH])
                nc.vector.tensor_add(ft[:, KT_D + do, :], f1[:], f2[:])
                yield

        def front(b):
            """Front-end of batch b. Yields 'c0' once chunk-0 features are
            complete (heavy work for the chunk may begin). Piece order is
            chosen so every vector/scalar dependency chain has PE matmuls
            emitted between its producer and its PE consumer."""
            st = state.setdefault(b, {})
            if "xt" not in st:  # batch 0's input DMAs were issued up front
                lxt = plx.tile([128, KT_D, LP], CDT, tag="lxt", name=f"lxt{b}")
                nc.sync.dma_start(lxt[:], lxin[b])
                st["lxt"] = lxt
                yield
                xt = px.tile([128, KT_D, LP], CDT, tag="xt", name=f"xt{b}")
                nc.sync.dma_start(xt[:], x_t[b])
                st["xt"] = xt
            xt = st["xt"]
            lxT = pfront.tile([128, KT_D, LP], CDT, tag="lxT", name=f"lxT{b}")
            st["lxT"] = lxT
            st["batt_cs"] = [pbatt.tile([128, 3, CH], CDT, tag="batt",
                                        name=f"batt{b}_{c}") for c in range(NCH)]
            st["feat_cs"] = [pfeat.tile([128, KT_2D, CH], CDT, tag="feat",
                                        name=f"feat{b}_{c}") for c in range(NCH)]
            # left halo of the window reads (never written by conv/LN)
            nc.gpsimd.memset(lxT[:, :, 0:1], 0.0)
            yield

            yield from conv_chunk(st, 0)
            yield from ln_stats(st, 0)
            yield from conv_chunk(st, 1)      # covers chunk-0 stat chain
            yield from ln_chain(st, 0)
            yield from ln_stats(st, 1)        # PE cover; scalar ops precede
            yield from ln_chain(st, 1)        # the ACT-heavy apply below
            yield from ln_bcast(st, 0)
            yield from ln_apply(st, 0)
            yield from smax_logits(st, 0)
            yield from ln_bcast(st, 1)
            yield from smax_den(st, 0)
            yield from ln_apply(st, 1)        # covers softmax denom chain
            # right pad: conv/LN wrote garbage over the zero-pad columns
            nc.gpsimd.memset(lxT[:, :, 1022:1026], 0.0)
            nc.gpsimd.memset(xt[:, :, 1022:1026], 0.0)
            yield from smax_bcast(st, 0)
            yield from win_chunk(st, 0)
            yield "c0"
            yield from smax_logits(st, 1)
            yield from smax_den(st, 1)
            yield from smax_bcast(st, 1)
            yield from win_chunk(st, 1)

        def heavy(b, fg, fg_pre=None):
            """SwiGLU+down+token of batch b. fg_pre (rest of this batch's own
            front) must be fully emitted before chunk 1 is used; fg (next
            batch's front) advances at interleave points."""
            pre = [fg_pre]

            def tick():
                if pre[0] is not None:
                    try:
                        next(pre[0])
                        return
                    except StopIteration:
                        pre[0] = None
                if fg is not None:
                    try:
                        next(fg)
                    except StopIteration:
                        pass

            def drain_pre():
                if pre[0] is not None:
                    for _ in pre[0]:
                        pass
                    pre[0] = None

            st = state[b]
            feat_cs = st["feat_cs"]
            y_b = py_sb.tile([128, LTB, D], CDT, tag="y_b", name=f"y_b{b}")
            h1s = []

            # pull the next front's input-DMA pieces right away so the
            # transfers run under this batch's compute
            tick()
            tick()

            # ---- SwiGLU: both chunks share each gate/up weight tile ----
            y_ps0 = [pyps.tile([128, D], fp32, tag="y", name=f"y0_{lt}")
                     for lt in range(4)]
            for i in range(NIT):
                gw, uw = guq.pop(0)
                fetch_gu()  # wraps into the next batch's first tiles
                hcs = []
                for c in range(NCH):
                    if c == 1:
                        drain_pre()  # chunk-1 features must be emitted
                    ft = feat_cs[c]
                    g_ps = pgen.tile([128, CH], fp32, tag="ps", name="g_ps")
                    for kt in range(KT_2D):
                        nc.tensor.matmul(g_ps[:], gw[:, kt, :], ft[:, kt, :],
                                         start=(kt == 0), stop=(kt == KT_2D - 1))
                    u_ps = pgen.tile([128, CH], fp32, tag="ps", name="u_ps")
                    for kt in range(KT_2D):
                        nc.tensor.matmul(u_ps[:], uw[:, kt, :], ft[:, kt, :],
                                         start=(kt == 0), stop=(kt == KT_2D - 1))
                    sg = psg.tile([128, CH], fp32, tag="sg", name="sg")
                    nc.scalar.activation(sg[:], g_ps[:], AF.Silu)
                    pool = ph0 if c == 0 else ph1
                    h = pool.tile([128, CH], CDT, tag=f"h{c}", name=f"h{c}_{i}")
                    nc.vector.tensor_mul(h[:], sg[:], u_ps[:])
                    hcs.append(h)
                # down for chunk 0 inline; chunk 1's h is kept for the tail
                h0 = hcs[0]
                for lt in range(4):
                    nc.tensor.matmul(y_ps0[lt][:], h0[:, lt * 128:(lt + 1) * 128],
                                     down_sb[:, i, :],
                                     start=(i == 0), stop=(i == NIT - 1))
                h1s.append(hcs[1])
                tick()
            for lt in range(4):
                if lt % 2 == 0:
                    nc.scalar.copy(y_b[:, lt, :], y_ps0[lt][:])
                else:
                    nc.vector.tensor_copy(y_b[:, lt, :], y_ps0[lt][:])
            tick()

            # ---- down tail for chunk 1 (pure PE block) ----
            # lt 0/1 accumulate in pgen's banks (idle here) so the tail does
            # not wait on chunk-0's PSUM->SBUF copies
            y_ps1 = [pgen.tile([128, D], fp32, tag="ps", name=f"y1_{lt}")
                     if lt < 2 else
                     pyps.tile([128, D], fp32, tag="y", name=f"y1_{lt}")
                     for lt in range(4)]
            for i in range(NIT):
                h1 = h1s[i]
                for lt in range(4):
                    nc.tensor.matmul(y_ps1[lt][:], h1[:, lt * 128:(lt + 1) * 128],
                                     down_sb[:, i, :],
                                     start=(i == 0), stop=(i == NIT - 1))
                if i % 4 == 3:
                    tick()
            for lt in range(4):
                if lt % 2 == 0:
                    nc.scalar.copy(y_b[:, 4 + lt, :], y_ps1[lt][:])
                else:
                    nc.vector.tensor_copy(y_b[:, 4 + lt, :], y_ps1[lt][:])
            tick()

            # ---- token_proj: out[m, d] = sum_l tok_w[l, m] * y[l, d] ----
            for m in range(MT):
                tw = twq.pop(0)
                fetch_tw()  # wraps into the next batch's first tiles
                t_ps = pyps.tile([128, D], fp32, tag="y", name="t_ps")
                for lt in range(LTB):
                    nc.tensor.matmul(t_ps[:], tw[:, lt, :], y_b[:, lt, :],
                                     start=(lt == 0), stop=(lt == LTB - 1))
                o_sb = pout.tile([128, D], fp32, tag="o_sb", name="o_sb")
                if m % 2 == 0:
                    nc.scalar.copy(o_sb[:], t_ps[:])
                else:
                    nc.vector.tensor_copy(o_sb[:], t_ps[:])
                nc.sync.dma_start(out_h[b, m * 128:(m + 1) * 128, :], o_sb[:])
                tick()
            # drain any remaining front pieces
            drain_pre()
            if fg is not None:
                for _ in fg:
                    pass

        def heavy0(fg, fg_pre):
            """Batch 0 only: the two chunks run SEQUENTIALLY (full SwiGLU
            over chunk 0, then chunk 1 with re-streamed weights). At kernel
            start the DVE still owes chunk 1's window mix; interleaving the
            chunks (as heavy() does) would stall every chunk-1 matmul behind
            that DVE backlog. The c0 pass gives the DVE ~60us of PE cover."""
            pre = [fg_pre]

            def tick():
                if pre[0] is not None:
                    try:
                        next(pre[0])
                        return
                    except StopIteration:
                        pre[0] = None
                if fg is not None:
                    try:
                        next(fg)
                    except StopIteration:
                        pass

            st = state[0]
            feat_cs = st["feat_cs"]
            y_b = py_sb.tile([128, LTB, D], CDT, tag="y_b", name="y_b0")

            for c in range(NCH):
                y_ps = [pyps.tile([128, D], fp32, tag="y", name=f"y{c}_{lt}")
                        for lt in range(4)]
                ft = feat_cs[c]
                for i in range(NIT):
                    gw, uw = guq.pop(0)
                    fetch_gu()
                    g_ps = pgen.tile([128, CH], fp32, tag="ps", name="g_ps")
                    for kt in range(KT_2D):
                        nc.tensor.matmul(g_ps[:], gw[:, kt, :], ft[:, kt, :],
                                         start=(kt == 0), stop=(kt == KT_2D - 1))
                    u_ps = pgen.tile([128, CH], fp32, tag="ps", name="u_ps")
                    for kt in range(KT_2D):
                        nc.tensor.matmul(u_ps[:], uw[:, kt, :], ft[:, kt, :],
                                         start=(kt == 0), stop=(kt == KT_2D - 1))
                    sg = psg.tile([128, CH], fp32, tag="sg", name="sg")
                    nc.scalar.activation(sg[:], g_ps[:], AF.Silu)
                    h = ph0.tile([128, CH], CDT, tag="h0", name=f"h_{c}_{i}")
                    nc.vector.tensor_mul(h[:], sg[:], u_ps[:])
                    for lt in range(4):
                        nc.tensor.matmul(y_ps[lt][:], h[:, lt * 128:(lt + 1) * 128],
                                         down_sb[:, i, :],
                                         start=(i == 0), stop=(i == NIT - 1))
                    tick()
                    tick()
                for lt in range(4):
                    if lt % 2 == 0:
                        nc.scalar.copy(y_b[:, 4 * c + lt, :], y_ps[lt][:])
                    else:
                        nc.vector.tensor_copy(y_b[:, 4 * c + lt, :], y_ps[lt][:])
                # chunk 1's features must be fully emitted before its pass
                if c == 0 and pre[0] is not None:
                    for _ in pre[0]:
                        pass
                    pre[0] = None

            for m in range(MT):
                tw = twq.pop(0)
                fetch_tw()
                t_ps = pyps.tile([128, D], fp32, tag="y", name="t_ps")
                for lt in range(LTB):
                    nc.tensor.matmul(t_ps[:], tw[:, lt, :], y_b[:, lt, :],
                                     start=(lt == 0), stop=(lt == LTB - 1))
                o_sb = pout.tile([128, D], fp32, tag="o_sb", name="o_sb")
                if m % 2 == 0:
                    nc.scalar.copy(o_sb[:], t_ps[:])
                else:
                    nc.vector.tensor_copy(o_sb[:], t_ps[:])
                nc.sync.dma_start(out_h[0, m * 128:(m + 1) * 128, :], o_sb[:])
                tick()
            if fg is not None:
                for _ in fg:
                    pass

        # gate/up and token weights are identical across batches, so the
        # prefetch queues wrap across heavy() calls: the next batch's
        # first tiles stream in during this batch's tail, removing the
        # weight-DMA stall at every batch/phase boundary. Batch 0 streams
        # the gate/up weights twice (once per chunk pass).
        guq, twq = [], []
        GU_TOT = (BPC + 1) * NIT
        gu_left = [GU_TOT]
        tw_left = [BPC * MT]

        def fetch_gu():
            if gu_left[0] <= 0:
                return
            i = (GU_TOT - gu_left[0]) % NIT
            gu_left[0] -= 1
            gw = pgu.tile([128, KT_2D, 128], CDT, tag="gw", name="gw")
            nc.sync.dma_start(gw[:], gate_wt[:, i])
            uw = pgu.tile([128, KT_2D, 128], CDT, tag="uw", name="uw")
            nc.sync.dma_start(uw[:], up_wt[:, i])
            guq.append((gw, uw))

        def fetch_tw():
            if tw_left[0] <= 0:
                return
            m = (BPC * MT - tw_left[0]) % MT
            tw_left[0] -= 1
            tw = ptw.tile([128, LTB, 128], CDT, tag="tw", name="tw")
            nc.sync.dma_start(tw[:], tok_wt[:, m])
            twq.append(tw)

        fetch_gu()
        fetch_gu()

        # software pipeline: batch 0's heavy starts at its chunk-0 features;
        # the rest of front(0) and front(b+1) interleave with heavy(b).
        # At startup there is no heavy work yet, so batch 1's conv matmuls
        # are interleaved into batch 0's front to cover the ~4us
        # single-partition RECIPROCAL chains (LN rstd / softmax denom).
        def pull(g, n=1):
            for _ in range(n):
                next(g)

        fg0 = front(0)
        fg1 = front(1)
        pull(fg0, 13)   # allocs, conv c0, stats c0, conv c1, both LN chains
        pull(fg1, 4)    # batch-1 input DMAs + conv c0 do0-1 (covers recips)
        # deepen the weight prefetch only after batch-1's input DMAs are
        # queued, so they don't delay the early PE cover
        for _ in range(2):
            fetch_gu()
        for _ in range(4):
            fetch_tw()
        pull(fg0, 3)    # LN bcast c0, apply c0, smax logits c0
        pull(fg1, 2)    # batch-1 conv c0 do2-3
        pull(fg0, 2)    # LN bcast c1, smax den c0 (reciprocal issued)
        pull(fg1, 2)    # batch-1 stats c0 + conv c1 do0 (covers den recip)
        pull(fg0, 2)    # apply c1, pad memsets + smax bcast c0
        for _ in range(3):
            pull(fg0, 1)   # win0 pieces interleaved with batch-1 conv
            pull(fg1, 1)
        pull(fg0, 1)    # win0 do3
        pull(fg0, 1)    # 'c0' marker
        pull(fg0, 1)    # smax logits c1
        pull(fg1, 1)
        pull(fg0, 1)    # smax den c1
        heavy0(fg1, fg0)   # fg_pre: smax bcast c1 + win1
        for b in range(1, BPC):
            fg = front(b + 1) if b + 1 < BPC else None
            heavy(b, fg)

    return nc


def _get_nc():
    global _NC_CACHE
    if _NC_CACHE is None:
        nc = bacc.Bacc("TRN2", target_bir_lowering=False, debug=False,
                       num_devices=NCORES)
        _emit(nc)
        nc.compile()
        nc.finalize()
        _NC_CACHE = nc
    return _NC_CACHE


def _prep_host(inputs):
    x = np.asarray(inputs["x"], np.float32)
    last_x = np.asarray(inputs["last_x"], np.float32)
    ln_g = np.asarray(inputs["ln_g"], np.float32)
    ln_b = np.asarray(inputs["ln_b"], np.float32)
    conv_w = np.asarray(inputs["conv_w"], np.float32)
    atten_w = np.asarray(inputs["atten_w"], np.float32)
    gate_w = np.asarray(inputs["gate_w"], np.float32)
    up_w = np.asarray(inputs["up_w"], np.float32)
    down_w = np.asarray(inputs["down_w"], np.float32)
    token_w = np.asarray(inputs["token_w"], np.float32)

    conv_a = np.ascontiguousarray(
        conv_w.transpose(1, 2, 0).reshape(KT_D, 128, 3, D).transpose(1, 0, 2, 3)
    ).astype(NPDT)
    atten_a = np.ascontiguousarray(
        atten_w.T.reshape(KT_D, 128, 3).transpose(1, 0, 2)).astype(NPDT)
    gate_a = np.ascontiguousarray(
        gate_w.T.reshape(KT_2D, 128, NIT, 128).transpose(1, 2, 0, 3)).astype(NPDT)
    up_a = np.ascontiguousarray(
        up_w.T.reshape(KT_2D, 128, NIT, 128).transpose(1, 2, 0, 3)).astype(NPDT)
    down_a = np.ascontiguousarray(
        down_w.T.reshape(NIT, 128, D).transpose(1, 0, 2)).astype(NPDT)
    twT = np.zeros((LTB * 128, MOUT), np.float32)
    twT[:L] = token_w.T
    tok_a = np.ascontiguousarray(
        twT.reshape(LTB, 128, MT, 128).transpose(1, 2, 0, 3)).astype(NPDT)
    g_a = np.ascontiguousarray(ln_g.reshape(KT_D, 128).T).astype(np.float32)
    b_a = np.ascontiguousarray(ln_b.reshape(KT_D, 128).T).astype(np.float32)

    def tr(t):  # [BPC, L, D] -> [BPC, 128, KT_D, LP] padded feature-major
        buf = np.zeros((BPC, D, LP), np.float32)
        buf[:, :, 1:1 + L] = t.transpose(0, 2, 1)
        return np.ascontiguousarray(
            buf.reshape(BPC, KT_D, 128, LP).transpose(0, 2, 1, 3)).astype(NPDT)

    in_maps = []
    for c in range(NCORES):
        s = slice(c * BPC, (c + 1) * BPC)
        in_maps.append({
            "x_t": tr(x[s]), "lx_t": tr(last_x[s]),
            "conv_wt": conv_a, "atten_wt": atten_a,
            "gate_wt": gate_a, "up_wt": up_a, "down_wt": down_a,
            "tok_wt": tok_a, "g_in": g_a, "b_in": b_a,
        })
    return in_maps


def kernel(**inputs):
    global LAST_EXEC_NS, LAST_RESULTS
    import os
    in_maps = _prep_host(inputs)
    nc = _get_nc()
    trace = bool(int(os.environ.get("KERNEL_TRACE", "0")))
    res = run_bass_kernel_spmd(nc, in_maps, core_ids=list(range(NCORES)),
                               trace=trace)
    LAST_EXEC_NS = res.exec_time_ns
    LAST_RESULTS = res.results
    return np.concatenate([r["out"] for r in res.results], axis=0)



# revision 30
# speedup vs baseline: 1.0140x; 1.0140x over previous
"""Trainium2 Bass kernel for nn_AttenBlock (sparse_attention).

Strategy: data-parallel over batch across 8 NeuronCores (4 batches/core).
Per core, a feature-major bf16 pipeline:
  LN(x) -> conv1d(last_x) -> softmax attention over 3-tap windows ->
  coarse/fine window mix -> SwiGLU -> down_proj -> token_proj.
All matmuls run on the PE array in bf16 (fp32 PSUM accumulation) with
moving free dim 512. Host pre-transposes activations/weights into the
SBUF layouts so no on-device transposes are needed.

Schedule: software-pipelined across batches (front of b+1 interleaves
with the SwiGLU/token phase of b). Within the front, conv runs first
(depends only on last_x) so the PE fills early at kernel start; for
batch 0 the heavy phase of chunk 0 starts as soon as chunk 0's features
are ready, while the rest of the front interleaves. The SwiGLU streams
gate/up weights once per i-tile shared by both 512-token chunks
(chunk 1's h tiles are kept in SBUF and its down_proj runs as a tail).
LayerNorm output is written in place over the x tile.

Token stream layout per batch: 1026 columns = [zero | 1021 tokens | 3
pad + zero edge]. Pad columns produce garbage tokens whose token_proj
weights are zero, so they never contribute to the output; the one halo
column each side of the real tokens is zeroed so windows match the
reference's zero padding.
"""

import sys

sys.path.insert(0, "/opt/trn_rl_repo")

import numpy as np
import ml_dtypes

import concourse.bass as bass
import concourse.mybir as mybir
import concourse.tile as tile
from concourse import bacc
from concourse.bass_utils import run_bass_kernel_spmd

# problem shapes (hardcoded; harness provides matching inputs)
B, L, D = 32, 1021, 512
INTER, MOUT = 2048, 4096
NCORES = 8
BPC = B // NCORES        # batches per core
KT_D = D // 128          # 4  feature k-tiles
KT_2D = 2 * KT_D         # 8  feat k-tiles
NIT = INTER // 128       # 16 inter tiles
LP = 1026                # padded token columns per batch
CH = 512                 # token chunk
NCH = 2                  # chunks per batch
LTB = 8                  # 128-token l-tiles per batch
MT = MOUT // 128         # 32 output m-tiles

fp32 = mybir.dt.float32
CDT = mybir.dt.bfloat16
NPDT = ml_dtypes.bfloat16
AF = mybir.ActivationFunctionType

LAST_EXEC_NS = None
LAST_RESULTS = None
_NC_CACHE = None


def _emit(nc):
    x_t = nc.dram_tensor("x_t", [BPC, 128, KT_D, LP], CDT, kind="ExternalInput")
    lxin = nc.dram_tensor("lx_t", [BPC, 128, KT_D, LP], CDT, kind="ExternalInput")
    conv_wt = nc.dram_tensor("conv_wt", [128, KT_D, 3, D], CDT, kind="ExternalInput")
    atten_wt = nc.dram_tensor("atten_wt", [128, KT_D, 3], CDT, kind="ExternalInput")
    gate_wt = nc.dram_tensor("gate_wt", [128, NIT, KT_2D, 128], CDT, kind="ExternalInput")
    up_wt = nc.dram_tensor("up_wt", [128, NIT, KT_2D, 128], CDT, kind="ExternalInput")
    down_wt = nc.dram_tensor("down_wt", [128, NIT, D], CDT, kind="ExternalInput")
    tok_wt = nc.dram_tensor("tok_wt", [128, MT, LTB, 128], CDT, kind="ExternalInput")
    g_in = nc.dram_tensor("g_in", [128, KT_D], fp32, kind="ExternalInput")
    b_in = nc.dram_tensor("b_in", [128, KT_D], fp32, kind="ExternalInput")
    out_h = nc.dram_tensor("out", [BPC, MOUT, D], fp32, kind="ExternalOutput")

    from contextlib import ExitStack
    from itertools import chain

    with tile.TileContext(nc) as tc, ExitStack() as ctx:
        singles = ctx.enter_context(tc.tile_pool(name="singles", bufs=1))
        px = ctx.enter_context(tc.tile_pool(name="px", bufs=2))
        plx = ctx.enter_context(tc.tile_pool(name="plx", bufs=1))
        pfront = ctx.enter_context(tc.tile_pool(name="pfront", bufs=2))
        pbatt = ctx.enter_context(tc.tile_pool(name="pbatt", bufs=2))
        pfeat = ctx.enter_context(tc.tile_pool(name="pfeat", bufs=4))
        pstatf = ctx.enter_context(tc.tile_pool(name="pstatf", bufs=4))
        pstatc = ctx.enter_context(tc.tile_pool(name="pstatc", bufs=4))
        pstatb = ctx.enter_context(tc.tile_pool(name="pstatb", bufs=2))
        psq = ctx.enter_context(tc.tile_pool(name="psq", bufs=2))
        ptmpc = ctx.enter_context(tc.tile_pool(name="ptmpc", bufs=2))
        pgu = ctx.enter_context(tc.tile_pool(name="pgu", bufs=4))
        psg = ctx.enter_context(tc.tile_pool(name="psg", bufs=2))
        ph0 = ctx.enter_context(tc.tile_pool(name="ph0", bufs=2))
        ph1 = ctx.enter_context(tc.tile_pool(name="ph1", bufs=NIT))
        ptw = ctx.enter_context(tc.tile_pool(name="ptw", bufs=4))
        py_sb = ctx.enter_context(tc.tile_pool(name="py_sb", bufs=1))
        pout = ctx.enter_context(tc.tile_pool(name="pout", bufs=2))
        pgen = ctx.enter_context(tc.tile_pool(name="pgen", bufs=2, space="PSUM"))
        pfps = ctx.enter_context(tc.tile_pool(name="pfps", bufs=2, space="PSUM"))
        pyps = ctx.enter_context(tc.tile_pool(name="pyps", bufs=4, space="PSUM"))

        # ---- batch-0 inputs FIRST: the DMA engines drain in emission order,
        # so the conv of batch 0 (first PE work) must not queue behind the
        # resident-weight transfers.
        state = {}
        st0 = state[0] = {}
        lxt0 = plx.tile([128, KT_D, LP], CDT, tag="lxt", name="lxt0")
        nc.sync.dma_start(lxt0[:], lxin[0])
        conv_sb = singles.tile([128, KT_D, 3, D], CDT)
        nc.sync.dma_start(conv_sb[:], conv_wt[:])
        xt0 = px.tile([128, KT_D, LP], CDT, tag="xt", name="xt0")
        nc.sync.dma_start(xt0[:], x_t[0])
        st0["xt"], st0["lxt"] = xt0, lxt0

        atten_sb = singles.tile([128, KT_D, 3], CDT)
        nc.sync.dma_start(atten_sb[:], atten_wt[:])
        g_sb = singles.tile([128, KT_D], fp32)
        nc.sync.dma_start(g_sb[:], g_in[:])
        b_sb = singles.tile([128, KT_D], fp32)
        nc.sync.dma_start(b_sb[:], b_in[:])
        down_sb = singles.tile([128, NIT, D], CDT)
        nc.sync.dma_start(down_sb[:], down_wt[:])
        ones_col = singles.tile([128, 1], CDT)
        nc.vector.memset(ones_col[:], 1.0)
        ones_row = singles.tile([1, 128], CDT)
        nc.vector.memset(ones_row[:], 1.0)
        eps_t = singles.tile([1, 1], fp32)
        nc.vector.memset(eps_t[:], 1e-5)

        def conv_chunk(st, c):
            """3-tap conv for one 512-col chunk, one do-tile per yield."""
            lxt, lxT = st["lxt"], st["lxT"]
            for do in range(KT_D):
                cps = pfps.tile([128, CH], fp32, tag="fps", name="cps")
                idx = 0
                for tap in range(3):
                    for kin in range(KT_D):
                        nc.tensor.matmul(
                            cps[:],
                            conv_sb[:, kin, tap, do * 128:(do + 1) * 128],
                            lxt[:, kin, c * CH + tap:c * CH + tap + CH],
                            start=(idx == 0), stop=(idx == 11))
                        idx += 1
                nc.vector.tensor_copy(lxT[:, do, 1 + c * CH:1 + c * CH + CH], cps[:])
                yield

        def ln_stats(st, c):
            """LayerNorm sum / sum-of-squares matmuls for one chunk."""
            xt = st["xt"]
            sl = slice(1 + c * CH, 1 + c * CH + CH)
            s1 = pfps.tile([128, CH], fp32, tag="fps", name="s1")
            for kt in range(KT_D):
                nc.tensor.matmul(s1[:1], ones_col[:], xt[:, kt, sl],
                                 start=(kt == 0), stop=(kt == KT_D - 1))
            s2 = pfps.tile([128, CH], fp32, tag="fps", name="s2")
            for kt in range(KT_D):
                sq = psq.tile([128, CH], CDT, tag="sq", name="sq")
                nc.vector.tensor_mul(sq[:], xt[:, kt, sl], xt[:, kt, sl])
                nc.tensor.matmul(s2[:1], ones_col[:], sq[:],
                                 start=(kt == 0), stop=(kt == KT_D - 1))
            st[f"s12_{c}"] = (s1, s2)
            yield

        def ln_chain(st, c):
            """mean/rstd computation (vector/scalar only, no PE)."""
            s1, s2 = st.pop(f"s12_{c}")
            mean_c = pstatc.tile([1, CH], CDT, tag="stc", name="mean_c")
            nc.vector.tensor_scalar_mul(mean_c[:], s1[:1], 1.0 / D)
            msq = pstatf.tile([1, CH], fp32, tag="stf", name="msq")
            nc.vector.tensor_mul(msq[:], mean_c[:], mean_c[:])
            var = pstatf.tile([1, CH], fp32, tag="stf", name="var")
            nc.vector.tensor_scalar_mul(var[:], s2[:1], 1.0 / D)
            nc.vector.tensor_sub(var[:], var[:], msq[:])
            # rstd via Ln/Exp tables (Rsqrt/Reciprocal ACT tables are
            # blocked for accuracy; single-partition DVE recip is ~4us)
            lnv = pstatf.tile([1, CH], fp32, tag="stf", name="lnv")
            nc.scalar.activation(lnv[:], var[:], AF.Ln, bias=eps_t[:])
            rstd_c = pstatc.tile([1, CH], CDT, tag="stc", name="rstd_c")
            nc.scalar.activation(rstd_c[:], lnv[:], AF.Exp, scale=-0.5)
            st[f"mr_{c}"] = (mean_c, rstd_c)
            yield

        def ln_bcast(st, c):
            """broadcast mean/rstd across partitions; copy out of PSUM
            eagerly so the PSUM ring is decoupled from the apply reads."""
            mean_c, rstd_c = st.pop(f"mr_{c}")
            mb_ps = pfps.tile([128, CH], fp32, tag="fps", name="mb_ps")
            nc.tensor.matmul(mb_ps[:], ones_row[:], mean_c[:], start=True, stop=True)
            rb_ps = pfps.tile([128, CH], fp32, tag="fps", name="rb_ps")
            nc.tensor.matmul(rb_ps[:], ones_row[:], rstd_c[:], start=True, stop=True)
            mb_sb = pstatb.tile([128, CH], CDT, tag="mb", name="mb_sb")
            nc.scalar.copy(mb_sb[:], mb_ps[:])
            rb_sb = pstatb.tile([128, CH], CDT, tag="rb", name="rb_sb")
            nc.scalar.copy(rb_sb[:], rb_ps[:])
            st[f"mrb_{c}"] = (mb_sb, rb_sb)
            yield

        def ln_apply(st, c):
            """xn = (x - m) * r * g + b, written over xt (vector/scalar)."""
            xt = st["xt"]
            mb_sb, rb_sb = st.pop(f"mrb_{c}")
            sl = slice(1 + c * CH, 1 + c * CH + CH)
            for kt in range(KT_D):
                t = ptmpc.tile([128, CH], CDT, tag="lnt", name="lnt")
                nc.vector.tensor_sub(t[:], xt[:, kt, sl], mb_sb[:])
                t2 = ptmpc.tile([128, CH], CDT, tag="lnt", name="lnt2")
                nc.vector.tensor_mul(t2[:], t[:], rb_sb[:])
                nc.scalar.activation(xt[:, kt, sl], t2[:], AF.Identity,
                                     bias=b_sb[:, kt:kt + 1],
                                     scale=g_sb[:, kt:kt + 1])
            yield

        def smax_logits(st, c):
            """attention logits + exp for one chunk (PE + scalar)."""
            lxT = st["lxT"]
            sl = slice(1 + c * CH, 1 + c * CH + CH)
            lg_ps = pfps.tile([128, CH], fp32, tag="fps", name="lg_ps")
            for kin in range(KT_D):
                nc.tensor.matmul(lg_ps[:3], atten_sb[:, kin, :],
                                 lxT[:, kin, sl],
                                 start=(kin == 0), stop=(kin == KT_D - 1))
            exp_sb = pstatc.tile([3, CH], CDT, tag="exp3", name="exp_sb")
            nc.scalar.activation(exp_sb[:], lg_ps[:3], AF.Exp)
            expk = [pstatc.tile([1, CH], CDT, tag="expk", name=f"expk{k}")
                    for k in range(3)]
            for k in range(3):
                nc.sync.dma_start(expk[k][:], exp_sb[k:k + 1, :])
            st[f"expk_{c}"] = (exp_sb, expk)
            yield

        def smax_den(st, c):
            """1/sum(exp): 3-partition matmul reduce + Ln/Exp tables."""
            exp_sb, _ = st[f"expk_{c}"]
            den_ps = pfps.tile([128, CH], fp32, tag="fps", name="den_ps")
            nc.tensor.matmul(den_ps[:1], ones_col[:3], exp_sb[:],
                             start=True, stop=True)
            lnd = pstatf.tile([1, CH], fp32, tag="stf", name="lnd")
            nc.scalar.activation(lnd[:], den_ps[:1], AF.Ln)
            rec_c = pstatc.tile([1, CH], CDT, tag="stc", name="rec_c")
            nc.scalar.activation(rec_c[:], lnd[:], AF.Exp, scale=-1.0)
            st[f"rec_{c}"] = rec_c
            yield

        def smax_bcast(st, c):
            """broadcast denom + exp taps across partitions -> batt."""
            _, expk = st.pop(f"expk_{c}")
            rec_c = st.pop(f"rec_{c}")
            brec_ps = pfps.tile([128, CH], fp32, tag="fps", name="brec_ps")
            nc.tensor.matmul(brec_ps[:], ones_row[:], rec_c[:], start=True, stop=True)
            brec_sb = ptmpc.tile([128, CH], CDT, tag="brec", name="brec_sb")
            nc.scalar.copy(brec_sb[:], brec_ps[:])
            bt = st["batt_cs"][c]
            for k in range(3):
                bex_ps = pfps.tile([128, CH], fp32, tag="fps", name="bex_ps")
                nc.tensor.matmul(bex_ps[:], ones_row[:], expk[k][:], start=True, stop=True)
                nc.vector.tensor_mul(bt[:, k, :], bex_ps[:], brec_sb[:])
            yield

        def win_chunk(st, c):
            """windowed coarse/fine mix for one chunk -> feat."""
            lxT, xt = st["lxT"], st["xt"]
            bt = st["batt_cs"][c]
            ft = st["feat_cs"][c]
            for do in range(KT_D):
                t1 = ptmpc.tile([128, CH], CDT, tag="w1", name="w1")
                t2 = ptmpc.tile([128, CH], CDT, tag="w2", name="w2")
                nc.vector.tensor_mul(t1[:], bt[:, 0, :], lxT[:, do, c * CH:c * CH + CH])
                nc.vector.tensor_mul(t2[:], bt[:, 1, :], lxT[:, do, c * CH + 1:c * CH + 1 + CH])
                nc.vector.tensor_add(t1[:], t1[:], t2[:])
                nc.vector.tensor_mul(t2[:], bt[:, 2, :], lxT[:, do, c * CH + 2:c * CH + 2 + CH])
                nc.vector.tensor_add(ft[:, do, :], t1[:], t2[:])
                f1 = ptmpc.tile([128, CH], CDT, tag="f1", name="f1")
                f2 = ptmpc.tile([128, CH], CDT, tag="f2", name="f2")
                nc.vector.tensor_mul(f1[:], bt[:, 0, :], xt[:, do, c * CH:c * CH + CH])
                nc.vector.tensor_mul(f2[:], bt[:, 1, :], xt[:, do, c * CH + 1:c * CH + 1 + CH])
                nc.vector.tensor_add(f1[:], f1[:], f2[:])
                nc.vector.tensor_mul(f2[:], bt[:, 2, :], xt[:, do, c * CH + 2:c * CH + 2 + CH])
                nc.vector.tensor_add(ft[:, KT_D + do, :], f1[:], f2[:])
                yield

        def front(b):
            """Front-end of batch b. Yields 'c0' once chunk-0 features are
            complete (heavy work for the chunk may begin). Piece order is
            chosen so every vector/scalar dependency chain has PE matmuls
            emitted between its producer and its PE consumer."""
            st = state.setdefault(b, {})
            if "xt" not in st:  # batch 0's input DMAs were issued up front
                lxt = plx.tile([128, KT_D, LP], CDT, tag="lxt", name=f"lxt{b}")
                nc.sync.dma_start(lxt[:], lxin[b])
                st["lxt"] = lxt
                yield
                xt = px.tile([128, KT_D, LP], CDT, tag="xt", name=f"xt{b}")
                nc.sync.dma_start(xt[:], x_t[b])
                st["xt"] = xt
            xt = st["xt"]
            lxT = pfront.tile([128, KT_D, LP], CDT, tag="lxT", name=f"lxT{b}")
            st["lxT"] = lxT
            st["batt_cs"] = [pbatt.tile([128, 3, CH], CDT, tag="batt",
                                        name=f"batt{b}_{c}") for c in range(NCH)]
            st["feat_cs"] = [pfeat.tile([128, KT_2D, CH], CDT, tag="feat",
                                        name=f"feat{b}_{c}") for c in range(NCH)]
            # left halo of the window reads (never written by conv/LN)
            nc.gpsimd.memset(lxT[:, :, 0:1], 0.0)
            yield

            yield from conv_chunk(st, 0)
            yield from ln_stats(st, 0)
            yield from conv_chunk(st, 1)      # covers chunk-0 stat chain
            yield from ln_chain(st, 0)
            yield from ln_stats(st, 1)        # PE cover; scalar ops precede
            yield from ln_chain(st, 1)        # the ACT-heavy apply below
            yield from ln_bcast(st, 0)
            yield from ln_apply(st, 0)
            yield from smax_logits(st, 0)
            yield from ln_bcast(st, 1)
            yield from smax_den(st, 0)
            yield from ln_apply(st, 1)        # covers softmax denom chain
            # right pad: conv/LN wrote garbage over the zero-pad columns
            nc.gpsimd.memset(lxT[:, :, 1022:1026], 0.0)
            nc.gpsimd.memset(xt[:, :, 1022:1026], 0.0)
            yield from smax_bcast(st, 0)
            yield from win_chunk(st, 0)
            yield "c0"
            yield from smax_logits(st, 1)
            yield from smax_den(st, 1)
            yield from smax_bcast(st, 1)
            yield from win_chunk(st, 1)

        def heavy(b, fg, fg_pre=None):
            """SwiGLU+down+token of batch b. fg_pre (rest of this batch's own
            front) must be fully emitted before chunk 1 is used; fg (next
            batch's front) advances at interleave points."""
            pre = [fg_pre]

            def tick():
                if pre[0] is not None:
                    try:
                        next(pre[0])
                        return
                    except StopIteration:
                        pre[0] = None
                if fg is not None:
                    try:
                        next(fg)
                    except StopIteration:
                        pass

            def drain_pre():
                if pre[0] is not None:
                    for _ in pre[0]:
                        pass
                    pre[0] = None

            st = state[b]
            feat_cs = st["feat_cs"]
            y_b = py_sb.tile([128, LTB, D], CDT, tag="y_b", name=f"y_b{b}")
            h1s = []

            # pull the next front's input-DMA pieces right away so the
            # transfers run under this batch's compute
            tick()
            tick()

            # ---- SwiGLU: both chunks share each gate/up weight tile ----
            y_ps0 = [pyps.tile([128, D], fp32, tag="y", name=f"y0_{lt}")
                     for lt in range(4)]
            for i in range(NIT):
                gw, uw = guq.pop(0)
                fetch_gu()  # wraps into the next batch's first tiles
                hcs = []
                for c in range(NCH):
                    if c == 1:
                        drain_pre()  # chunk-1 features must be emitted
                    ft = feat_cs[c]
                    g_ps = pgen.tile([128, CH], fp32, tag="ps", name="g_ps")
                    for kt in range(KT_2D):
                        nc.tensor.matmul(g_ps[:], gw[:, kt, :], ft[:, kt, :],
                                         start=(kt == 0), stop=(kt == KT_2D - 1))
                    u_ps = pgen.tile([128, CH], fp32, tag="ps", name="u_ps")
                    for kt in range(KT_2D):
                        nc.tensor.matmul(u_ps[:], uw[:, kt, :], ft[:, kt, :],
                                         start=(kt == 0), stop=(kt == KT_2D - 1))
                    sg = psg.tile([128, CH], fp32, tag="sg", name="sg")
                    nc.scalar.activation(sg[:], g_ps[:], AF.Silu)
                    pool = ph0 if c == 0 else ph1
                    h = pool.tile([128, CH], CDT, tag=f"h{c}", name=f"h{c}_{i}")
                    nc.vector.tensor_mul(h[:], sg[:], u_ps[:])
                    hcs.append(h)
                # down for chunk 0 inline; chunk 1's h is kept for the tail
                h0 = hcs[0]
                for lt in range(4):
                    nc.tensor.matmul(y_ps0[lt][:], h0[:, lt * 128:(lt + 1) * 128],
                                     down_sb[:, i, :],
                                     start=(i == 0), stop=(i == NIT - 1))
                h1s.append(hcs[1])
                tick()
            for lt in range(4):
                if lt % 2 == 0:
                    nc.scalar.copy(y_b[:, lt, :], y_ps0[lt][:])
                else:
                    nc.vector.tensor_copy(y_b[:, lt, :], y_ps0[lt][:])
            tick()

            # ---- down tail for chunk 1 (pure PE block) ----
            # lt 0/1 accumulate in pgen's banks (idle here) so the tail does
            # not wait on chunk-0's PSUM->SBUF copies
            y_ps1 = [pgen.tile([128, D], fp32, tag="ps", name=f"y1_{lt}")
                     if lt < 2 else
                     pyps.tile([128, D], fp32, tag="y", name=f"y1_{lt}")
                     for lt in range(4)]
            for i in range(NIT):
                h1 = h1s[i]
                for lt in range(4):
                    nc.tensor.matmul(y_ps1[lt][:], h1[:, lt * 128:(lt + 1) * 128],
                                     down_sb[:, i, :],
                                     start=(i == 0), stop=(i == NIT - 1))
                if i % 4 == 3:
                    tick()
            for lt in range(4):
                if lt % 2 == 0:
                    nc.scalar.copy(y_b[:, 4 + lt, :], y_ps1[lt][:])
                else:
                    nc.vector.tensor_copy(y_b[:, 4 + lt, :], y_ps1[lt][:])
            tick()

            # ---- token_proj: out[m, d] = sum_l tok_w[l, m] * y[l, d] ----
            for m in range(MT):
                tw = twq.pop(0)
                fetch_tw()  # wraps into the next batch's first tiles
                t_ps = pyps.tile([128, D], fp32, tag="y", name="t_ps")
                for lt in range(LTB):
                    nc.tensor.matmul(t_ps[:], tw[:, lt, :], y_b[:, lt, :],
                                     start=(lt == 0), stop=(lt == LTB - 1))
                o_sb = pout.tile([128, D], fp32, tag="o_sb", name="o_sb")
                if m % 2 == 0:
                    nc.scalar.copy(o_sb[:], t_ps[:])
                else:
                    nc.vector.tensor_copy(o_sb[:], t_ps[:])
                nc.sync.dma_start(out_h[b, m * 128:(m + 1) * 128, :], o_sb[:])
                tick()
            # drain any remaining front pieces
            drain_pre()
            if fg is not None:
                for _ in fg:
                    pass

        def heavy0(fg, fg_pre):
            """Batch 0 only: the two chunks run SEQUENTIALLY (full SwiGLU
            over chunk 0, then chunk 1 with re-streamed weights). At kernel
            start the DVE still owes chunk 1's window mix; interleaving the
            chunks (as heavy() does) would stall every chunk-1 matmul behind
            that DVE backlog. The c0 pass gives the DVE ~60us of PE cover."""
            pre = [fg_pre]

            def tick():
                if pre[0] is not None:
                    try:
                        next(pre[0])
                        return
                    except StopIteration:
                        pre[0] = None
                if fg is not None:
                    try:
                        next(fg)
                    except StopIteration:
                        pass

            st = state[0]
            feat_cs = st["feat_cs"]
            y_b = py_sb.tile([128, LTB, D], CDT, tag="y_b", name="y_b0")

            for c in range(NCH):
                y_ps = [pyps.tile([128, D], fp32, tag="y", name=f"y{c}_{lt}")
                        for lt in range(4)]
                ft = feat_cs[c]
                for i in range(NIT):
                    gw, uw = guq.pop(0)
                    fetch_gu()
                    g_ps = pgen.tile([128, CH], fp32, tag="ps", name="g_ps")
                    for kt in range(KT_2D):
                        nc.tensor.matmul(g_ps[:], gw[:, kt, :], ft[:, kt, :],
                                         start=(kt == 0), stop=(kt == KT_2D - 1))
                    u_ps = pgen.tile([128, CH], fp32, tag="ps", name="u_ps")
                    for kt in range(KT_2D):
                        nc.tensor.matmul(u_ps[:], uw[:, kt, :], ft[:, kt, :],
                                         start=(kt == 0), stop=(kt == KT_2D - 1))
                    sg = psg.tile([128, CH], fp32, tag="sg", name="sg")
                    nc.scalar.activation(sg[:], g_ps[:], AF.Silu)
                    h = ph0.tile([128, CH], CDT, tag="h0", name=f"h_{c}_{i}")
                    nc.vector.tensor_mul(h[:], sg[:], u_ps[:])
                    for lt in range(4):
                        nc.tensor.matmul(y_ps[lt][:], h[:, lt * 128:(lt + 1) * 128],
                                         down_sb[:, i, :],
                                         start=(i == 0), stop=(i == NIT - 1))
                    tick()
                    tick()
                for lt in range(4):
                    if lt % 2 == 0:
                        nc.scalar.copy(y_b[:, 4 * c + lt, :], y_ps[lt][:])
                    else:
                        nc.vector.tensor_copy(y_b[:, 4 * c + lt, :], y_ps[lt][:])
                # chunk 1's features must be fully emitted before its pass
                if c == 0 and pre[0] is not None:
                    for _ in pre[0]:
                        pass
                    pre[0] = None

            for m in range(MT):
                tw = twq.pop(0)
                fetch_tw()
                t_ps = pyps.tile([128, D], fp32, tag="y", name="t_ps")
                for lt in range(LTB):
                    nc.tensor.matmul(t_ps[:], tw[:, lt, :], y_b[:, lt, :],
                                     start=(lt == 0), stop=(lt == LTB - 1))
                o_sb = pout.tile([128, D], fp32, tag="o_sb", name="o_sb")
                if m % 2 == 0:
                    nc.scalar.copy(o_sb[:], t_ps[:])
                else:
                    nc.vector.tensor_copy(o_sb[:], t_ps[:])
                nc.sync.dma_start(out_h[0, m * 128:(m + 1) * 128, :], o_sb[:])
                tick()
            if fg is not None:
                for _ in fg:
                    pass

        # gate/up and token weights are identical across batches, so the
        # prefetch queues wrap across heavy() calls: the next batch's
        # first tiles stream in during this batch's tail, removing the
        # weight-DMA stall at every batch/phase boundary. Batch 0 streams
        # the gate/up weights twice (once per chunk pass).
        guq, twq = [], []
        GU_TOT = (BPC + 1) * NIT
        gu_left = [GU_TOT]
        tw_left = [BPC * MT]

        def fetch_gu():
            if gu_left[0] <= 0:
                return
            i = (GU_TOT - gu_left[0]) % NIT
            gu_left[0] -= 1
            gw = pgu.tile([128, KT_2D, 128], CDT, tag="gw", name="gw")
            nc.sync.dma_start(gw[:], gate_wt[:, i])
            uw = pgu.tile([128, KT_2D, 128], CDT, tag="uw", name="uw")
            nc.sync.dma_start(uw[:], up_wt[:, i])
            guq.append((gw, uw))

        def fetch_tw():
            if tw_left[0] <= 0:
                return
            m = (BPC * MT - tw_left[0]) % MT
            tw_left[0] -= 1
            tw = ptw.tile([128, LTB, 128], CDT, tag="tw", name="tw")
            nc.sync.dma_start(tw[:], tok_wt[:, m])
            twq.append(tw)

        fetch_gu()
        fetch_gu()

        # software pipeline: batch 0's heavy starts at its chunk-0 features;
        # the rest of front(0) and front(b+1) interleave with heavy(b).
        # At startup there is no heavy work yet, so batch 1's conv matmuls
        # are interleaved into batch 0's front to cover the ~4us
        # single-partition RECIPROCAL chains (LN rstd / softmax denom).
        def pull(g, n=1):
            for _ in range(n):
                next(g)

        fg0 = front(0)
        fg1 = front(1)
        pull(fg0, 13)   # allocs, conv c0, stats c0, conv c1, both LN chains
        pull(fg1, 4)    # batch-1 input DMAs + conv c0 do0-1 (covers recips)
        # deepen the weight prefetch only after batch-1's input DMAs are
        # queued, so they don't delay the early PE cover
        for _ in range(2):
            fetch_gu()
        for _ in range(4):
            fetch_tw()
        pull(fg0, 3)    # LN bcast c0, apply c0, smax logits c0
        pull(fg1, 2)    # batch-1 conv c0 do2-3
        pull(fg0, 2)    # LN bcast c1, smax den c0 (reciprocal issued)
        pull(fg1, 2)    # batch-1 stats c0 + conv c1 do0 (covers den recip)
        pull(fg0, 2)    # apply c1, pad memsets + smax bcast c0
        for _ in range(3):
            pull(fg0, 1)   # win0 pieces interleaved with batch-1 conv
            pull(fg1, 1)
        pull(fg0, 1)    # win0 do3
        pull(fg0, 1)    # 'c0' marker
        pull(fg0, 1)    # smax logits c1
        pull(fg1, 1)
        pull(fg0, 1)    # smax den c1
        heavy0(fg1, fg0)   # fg_pre: smax bcast c1 + win1
        for b in range(1, BPC):
            fg = front(b + 1) if b + 1 < BPC else None
            heavy(b, fg)

    return nc


def _get_nc():
    global _NC_CACHE
    if _NC_CACHE is None:
        nc = bacc.Bacc("TRN2", target_bir_lowering=False, debug=False,
                       num_devices=NCORES)
        _emit(nc)
        nc.compile()
        nc.finalize()
        _NC_CACHE = nc
    return _NC_CACHE


def _prep_host(inputs):
    x = np.asarray(inputs["x"], np.float32)
    last_x = np.asarray(inputs["last_x"], np.float32)
    ln_g = np.asarray(inputs["ln_g"], np.float32)
    ln_b = np.asarray(inputs["ln_b"], np.float32)
    conv_w = np.asarray(inputs["conv_w"], np.float32)
    atten_w = np.asarray(inputs["atten_w"], np.float32)
    gate_w = np.asarray(inputs["gate_w"], np.float32)
    up_w = np.asarray(inputs["up_w"], np.float32)
    down_w = np.asarray(inputs["down_w"], np.float32)
    token_w = np.asarray(inputs["token_w"], np.float32)

    conv_a = np.ascontiguousarray(
        conv_w.transpose(1, 2, 0).reshape(KT_D, 128, 3, D).transpose(1, 0, 2, 3)
    ).astype(NPDT)
    atten_a = np.ascontiguousarray(
        atten_w.T.reshape(KT_D, 128, 3).transpose(1, 0, 2)).astype(NPDT)
    gate_a = np.ascontiguousarray(
        gate_w.T.reshape(KT_2D, 128, NIT, 128).transpose(1, 2, 0, 3)).astype(NPDT)
    up_a = np.ascontiguousarray(
        up_w.T.reshape(KT_2D, 128, NIT, 128).transpose(1, 2, 0, 3)).astype(NPDT)
    down_a = np.ascontiguousarray(
        down_w.T.reshape(NIT, 128, D).transpose(1, 0, 2)).astype(NPDT)
    twT = np.zeros((LTB * 128, MOUT), np.float32)
    twT[:L] = token_w.T
    tok_a = np.ascontiguousarray(
        twT.reshape(LTB, 128, MT, 128).transpose(1, 2, 0, 3)).astype(NPDT)
    g_a = np.ascontiguousarray(ln_g.reshape(KT_D, 128).T).astype(np.float32)
    b_a = np.ascontiguousarray(ln_b.reshape(KT_D, 128).T).astype(np.float32)

    def tr(t):  # [BPC, L, D] -> [BPC, 128, KT_D, LP] padded feature-major
        buf = np.zeros((BPC, D, LP), np.float32)
        buf[:, :, 1:1 + L] = t.transpose(0, 2, 1)
        return np.ascontiguousarray(
            buf.reshape(BPC, KT_D, 128, LP).transpose(0, 2, 1, 3)).astype(NPDT)

    in_maps = []
    for c in range(NCORES):
        s = slice(c * BPC, (c + 1) * BPC)
        in_maps.append({
            "x_t": tr(x[s]), "lx_t": tr(last_x[s]),
            "conv_wt": conv_a, "atten_wt": atten_a,
            "gate_wt": gate_a, "up_wt": up_a, "down_wt": down_a,
            "tok_wt": tok_a, "g_in": g_a, "b_in": b_a,
        })
    return in_maps


def kernel(**inputs):
    global LAST_EXEC_NS, LAST_RESULTS
    import os
    in_maps = _prep_host(inputs)
    nc = _get_nc()
    trace = bool(int(os.environ.get("KERNEL_TRACE", "0")))
    res = run_bass_kernel_spmd(nc, in_maps, core_ids=list(range(NCORES)),
                               trace=trace)
    LAST_EXEC_NS = res.exec_time_ns
    LAST_RESULTS = res.results
    return np.concatenate([r["out"] for r in res.results], axis=0)

